# revision 21
# baseline (speedup 1.0000x reference)
"""Multi-head attention (B=2, N=2048, D=1024, H=16) on 8 Trainium2 cores.

Sharding: data-parallel over batch (cores 0-3 -> b=0, cores 4-7 -> b=1) and
tensor-parallel over heads (4 heads per core = 256 of 1024 QKV/O channels).
Each core computes its 4 heads' attention plus a partial output projection;
the host sums the 4 partials per batch and adds bo.

v2 pipeline (per core):
 - Input DMA spread over 4 engine queues; projections run chunk-major in a
   dedicated pre-phase PSUM pool so each weight-chunk matmul fires as soon
   as its xT d-chunk lands.
 - Attention processes a (pair, 512-query-chunk) block at a time.  Per
   k-iter the TWO heads of the pair run their scores matmuls CONCURRENTLY
   in disjoint PE row-groups (K=64 each, tile_position (0,0)/(64,0)) into
   the two halves of one [128,1024] PSUM tile; a single FD=1024 exp on
   ScalarE covers both heads; PV (M=65 with the ones/denominator column)
   runs per head with a one-iter lag like the baseline.
 - Normalization reads PSUM directly: reciprocal_approx_fast on the den
   row, GPSIMD partition_broadcast, one tensor_mul into at.
 - qc-outer / pair-inner block order lets oproj units for query chunk qc
   run as PE filler work two blocks later; only the last chunk's oproj
   trails the attention.
"""

import numpy as np

import concourse.bass as bass
import concourse.bacc as bacc
import concourse.tile as tile
from concourse import mybir
from concourse.bass_utils import run_bass_kernel_spmd

F32 = mybir.dt.float32
BF16 = mybir.dt.bfloat16
AF = mybir.ActivationFunctionType

B, N, D, H, HD = 2, 2048, 1024, 16, 64
E = 256            # channels per core (4 heads * 64)
DC = D // 128      # 8 contraction chunks for projections
NB = N // 128      # 16 token blocks / k chunks
QC = 512           # query chunk
NQC = N // QC      # 4 query chunks
SCALE = 1.0 / np.sqrt(HD)
DT = BF16


def _emit(nc):
    xT = nc.dram_tensor("xT", [D, N], DT, kind="ExternalInput")
    wqT = nc.dram_tensor("wqT", [D, E], DT, kind="ExternalInput")
    wkT = nc.dram_tensor("wkT", [D, E], DT, kind="ExternalInput")
    wvT = nc.dram_tensor("wvT", [D, E], DT, kind="ExternalInput")
    woT = nc.dram_tensor("woT", [E, D], DT, kind="ExternalInput")
    bq2 = nc.dram_tensor("bq2", [128, 2], F32, kind="ExternalInput")
    bk2 = nc.dram_tensor("bk2", [128, 2], F32, kind="ExternalInput")
    bv1 = nc.dram_tensor("bv1", [E], F32, kind="ExternalInput")
    vones = nc.dram_tensor("vones", [128, NB, 4], DT, kind="ExternalInput")
    out = nc.dram_tensor("out", [N, D], F32, kind="ExternalOutput")

    with tile.TileContext(nc) as tc:
        with tc.tile_pool(name="per", bufs=1) as per, \
             tc.tile_pool(name="wp", bufs=12) as wp, \
             tc.tile_pool(name="dn", bufs=2) as dn, \
             tc.tile_pool(name="up", bufs=2) as up, \
             tc.tile_pool(name="op", bufs=4) as op:

            # ---- persistent SBUF tiles ----
            # xT loads as 3 big parallel DMAs (one per DMA ring) into 3
            # tiles; W matrices load per-chunk.  Separate tiles per DMA so
            # no false write-order deps serialize the transfers.
            xts = [per.tile([128, 3, N], DT, name="xta"),
                   per.tile([128, 3, N], DT, name="xtb"),
                   per.tile([128, 2, N], DT, name="xtc")]
            xt = [xts[dc // 3][:, dc % 3, :] for dc in range(DC)]
            wq = [per.tile([128, E], DT, name=f"wq{i}") for i in range(DC)]
            wk = [per.tile([128, E], DT, name=f"wk{i}") for i in range(DC)]
            wv = [per.tile([128, E], DT, name=f"wv{i}") for i in range(DC)]
            wo = per.tile([128, 2, D], DT)            # WoT (e-chunk)
            qt = per.tile([128, 2, N], DT)            # Q^T: (pair, tokens)
            kt = per.tile([128, 2, N], DT)
            vp = per.tile([128, NB, 4, 128], DT)      # V natural + ones col
            at = per.tile([128, 2, N], DT)            # attn^T normalized
            bqs = per.tile([128, 2], F32)
            bks = per.tile([128, 2], F32)
            bvb = per.tile([128, E], F32)

            # ---- input DMA: few big transfers, 3 rings in parallel ----
            # sync ring: wk chunks 0-3, xT third 0, wk 4-7, biases
            # scalar ring: wq 0-3, xT third 1, wq 4-7, vones
            # gpsimd ring: wv 0-3, xT third 2, wv 4-7, bvb, wo
            qs = [nc.sync, nc.scalar, nc.gpsimd]
            for dc in range(4):
                nc.sync.dma_start(out=wk[dc], in_=wkT[dc * 128:(dc + 1) * 128, :])
                nc.scalar.dma_start(out=wq[dc], in_=wqT[dc * 128:(dc + 1) * 128, :])
                nc.gpsimd.dma_start(out=wv[dc], in_=wvT[dc * 128:(dc + 1) * 128, :])
            nc.sync.dma_start(out=xts[0], in_=xT[0:384, :].rearrange(
                "(c p) n -> p c n", p=128))
            nc.scalar.dma_start(out=xts[1], in_=xT[384:768, :].rearrange(
                "(c p) n -> p c n", p=128))
            nc.gpsimd.dma_start(out=xts[2], in_=xT[768:1024, :].rearrange(
                "(c p) n -> p c n", p=128))
            for dc in range(4, DC):
                nc.sync.dma_start(out=wk[dc], in_=wkT[dc * 128:(dc + 1) * 128, :])
                nc.scalar.dma_start(out=wq[dc], in_=wqT[dc * 128:(dc + 1) * 128, :])
                nc.gpsimd.dma_start(out=wv[dc], in_=wvT[dc * 128:(dc + 1) * 128, :])
            nc.sync.dma_start(out=bqs, in_=bq2[:, :])
            nc.sync.dma_start(out=bks, in_=bk2[:, :])
            bv_ap = bv1[:]
            nc.gpsimd.dma_start(
                out=bvb,
                in_=bass.AP(tensor=bv_ap.tensor, offset=0, ap=[[0, 128], [1, E]]),
            )
            nc.scalar.dma_start(out=vp[:, :, :, HD:HD + 1],
                                in_=vones[:, :, :].rearrange(
                                    "p a (b o) -> p a b o", o=1))
            for ec in range(2):
                nc.gpsimd.dma_start(out=wo[:, ec, :],
                                    in_=woT[ec * 128:(ec + 1) * 128, :])

            # ---- pre-phase: warmup + chunk-major first projections ----
            # K0 g0-3, Q0 g0, V nb0-1 accumulate concurrently in a dedicated
            # PSUM pool (7 banks); each d-chunk's matmuls fire as the chunk
            # arrives from HBM.
            with tc.tile_pool(name="pre", bufs=1, space="PSUM") as pre:
                pk = [pre.tile([128, 512], F32, tag=f"p{g}", name=f"pk{g}")
                      for g in range(4)]
                pq0 = pre.tile([128, 512], F32, tag="p4", name="pq0")
                pv01 = [pre.tile([128, E], F32, tag=f"p{5 + i}", name=f"pv{i}")
                        for i in range(2)]
                for dc in range(DC):
                    for g in range(4):
                        nc.tensor.matmul(
                            pk[g], wk[dc][:, 0:128],
                            xt[dc][:, g * 512:(g + 1) * 512],
                            start=(dc == 0), stop=(dc == DC - 1))
                    nc.tensor.matmul(
                        pq0, wq[dc][:, 0:128], xt[dc][:, 0:512],
                        start=(dc == 0), stop=(dc == DC - 1))
                    for i in range(2):
                        nc.tensor.matmul(
                            pv01[i], xt[dc][:, i * 128:(i + 1) * 128],
                            wv[dc],
                            start=(dc == 0), stop=(dc == DC - 1))
                for g in range(4):
                    nc.vector.tensor_scalar_add(
                        kt[:, 0, g * 512:(g + 1) * 512], pk[g], bks[:, 0:1])
                nc.vector.tensor_scalar_add(qt[:, 0, 0:512], pq0, bqs[:, 0:1])
                for i in range(2):
                    nc.vector.tensor_add(
                        vp[:, i, :, 0:HD],
                        pv01[i].rearrange("p (h d) -> p h d", h=4),
                        bvb.rearrange("p (h d) -> p h d", h=4))

            with tc.tile_pool(name="ps", bufs=1, space="PSUM") as ps:
                pj_n = [0]

                def pj_tag():
                    pj_n[0] += 1
                    return ("pjA", "pjB")[pj_n[0] % 2]

                # ---- filler units (1-bank psum groups on pj tags) ----
                # K/Q projection groups run at boosted priority: they feed
                # the NEXT block's scores and must not queue behind the
                # V-projection / PV backlog.
                def proj_group(wsb, dst, bias, pair, n4):
                    def emit():
                        with tc.high_priority(offset=1 << 19):
                            pt = ps.tile([128, 512], F32, tag=pj_tag(),
                                         name="ppj")
                            for dc in range(DC):
                                nc.tensor.matmul(
                                    pt[:, :],
                                    wsb[dc][:, pair * 128:(pair + 1) * 128],
                                    xt[dc][:, n4 * 512:(n4 + 1) * 512],
                                    start=(dc == 0), stop=(dc == DC - 1))
                            nc.vector.tensor_scalar_add(
                                dst[:, pair, n4 * 512:(n4 + 1) * 512], pt[:, :],
                                bias[:, pair:pair + 1])
                    return emit

                def vnat_group(nb):
                    def emit():
                        pt = ps.tile([128, E], F32, tag=pj_tag(), name="pvn")
                        for dc in range(DC):
                            nc.tensor.matmul(
                                pt[:, :],
                                xt[dc][:, nb * 128:(nb + 1) * 128],
                                wv[dc],
                                start=(dc == 0), stop=(dc == DC - 1))
                        nc.vector.tensor_add(
                            vp[:, nb, :, 0:HD],
                            pt.rearrange("p (h d) -> p h d", h=4),
                            bvb.rearrange("p (h d) -> p h d", h=4))
                    return emit

                o_n = [0]

                def oproj_unit(nb, evict="dve", tags=None):
                    # both D-halves of a 128-token block -> one contiguous
                    # [128, 1024] row store
                    def emit():
                        ot = op.tile([128, 1024], F32, tag="ot", name="ot")
                        for half in range(2):
                            po = ps.tile([128, 512], F32,
                                         tag=(tags[half] if tags
                                              else pj_tag()),
                                         name="po")
                            for ec in range(2):
                                nc.tensor.matmul(
                                    po[:, :],
                                    at[:, ec, nb * 128:(nb + 1) * 128],
                                    wo[:, ec, half * 512:(half + 1) * 512],
                                    start=(ec == 0), stop=(ec == 1))
                            if evict == "dve":
                                nc.vector.tensor_copy(
                                    ot[:, half * 512:(half + 1) * 512], po)
                            else:
                                nc.scalar.copy(
                                    ot[:, half * 512:(half + 1) * 512], po)
                        o_n[0] += 1
                        qs[o_n[0] % 3].dma_start(
                            out=out[nb * 128:(nb + 1) * 128, :], in_=ot)
                    return emit

                # ---- one (pair, qc) attention block: 16 k-iters ----
                # Returns a `finish` closure (last PV pair + normalization);
                # the caller runs it after the NEXT block's first k-iter so
                # ScalarE never stalls across block boundaries.
                def attn_block(pair, qc, fillers, carry=None):
                    q0 = qc * QC
                    fi = 0
                    pvs = [ps.tile([HD + 1, QC], F32, tag=t, name=t)
                           for t in ("pvA", "pvB")]
                    wtiles = {}
                    for k in range(NB):
                        st = ps.tile([128, 1024], F32,
                                     tag=("s0", "s1")[k % 2], name="st")
                        # scores for both heads back-to-back at max priority
                        # so they sit adjacent in the PE queue and overlap in
                        # disjoint row-groups of the array.
                        with tc.high_priority(offset=1 << 20):
                            for hh in range(2):
                                p0 = hh * HD
                                nc.tensor.matmul(
                                    st[:, hh * QC:(hh + 1) * QC],
                                    kt[p0:p0 + HD, pair, k * 128:(k + 1) * 128],
                                    qt[p0:p0 + HD, pair, q0:q0 + QC],
                                    start=True, stop=True,
                                    tile_position=(p0, 0))
                        w = wp.tile([128, 1024], DT, tag="w", name="w")
                        nc.scalar.activation(w, st, AF.Exp, scale=SCALE)
                        wtiles[k] = w
                        if k == 0 and carry is not None:
                            carry()
                            carry = None
                        while fi < (k + 1) * len(fillers) // NB:
                            fillers[fi]()
                            fi += 1
                        if k > 0:
                            wprev = wtiles.pop(k - 1)
                            for hh in range(2):
                                nc.tensor.matmul(
                                    pvs[hh][:, :],
                                    vp[:, k - 1, 2 * pair + hh, 0:HD + 1],
                                    wprev[:, hh * QC:(hh + 1) * QC],
                                    start=(k - 1 == 0), stop=False)
                    wlast = wtiles.pop(NB - 1)

                    def finish():
                        for hh in range(2):
                            nc.tensor.matmul(
                                pvs[hh][:, :],
                                vp[:, NB - 1, 2 * pair + hh, 0:HD + 1],
                                wlast[:, hh * QC:(hh + 1) * QC],
                                start=False, stop=True)
                        # normalize both heads, ops interleaved so the GP
                        # broadcasts overlap the DVE chain; the at-mul reads
                        # pv straight from PSUM.
                        den = [dn.tile([1, QC], F32, tag=f"den{h}",
                                       name=f"den{h}") for h in range(2)]
                        rec = [dn.tile([1, QC], F32, tag=f"rec{h}",
                                       name=f"rec{h}") for h in range(2)]
                        bcr = [up.tile([HD, QC], F32, tag=f"bcr{h}",
                                       name=f"bcr{h}") for h in range(2)]
                        for hh in range(2):
                            nc.vector.tensor_copy(den[hh],
                                                  pvs[hh][HD:HD + 1, :])
                        for hh in range(2):
                            nc.vector.reciprocal_approx_fast(rec[hh], den[hh])
                            nc.gpsimd.partition_broadcast(bcr[hh],
                                                          rec[hh][0:1, :])
                        for hh in range(2):
                            p0 = hh * HD
                            nc.vector.tensor_mul(
                                at[p0:p0 + HD, pair, q0:q0 + QC],
                                pvs[hh][0:HD, :], bcr[hh])
                    return finish

                # ---- filler inventory ----
                V = [vnat_group(i) for i in range(NB)]
                K1 = [proj_group(wk, kt, bks, 1, g) for g in range(4)]
                Q0 = [proj_group(wq, qt, bqs, 0, g) for g in range(4)]
                Q1 = [proj_group(wq, qt, bqs, 1, g) for g in range(4)]
                # tail units (qc=3) run after attention: they can rotate
                # over the freed attention PSUM banks and use ScalarE for
                # half the evictions.
                TT = [("pjA", "pjB"), ("pvA", "pvB"), ("s0", "s1")]
                O = [[oproj_unit(4 * qc + nb,
                                 evict=("dve" if qc < 3 else
                                        ("dve", "act")[nb % 2]),
                                 tags=(None if qc < 3 else TT[nb % 3]))
                      for nb in range(4)] for qc in range(NQC)]

                # V[nb] feeds PV at iter nb of the qc=0 blocks; keep a
                # 2-iter lead.  K1 g must land before block (1,0) iter 4g.
                sched = [
                    (0, 0, [K1[0], V[2], V[3], V[4], V[5], V[6], V[7],
                            V[8], V[9], V[10], V[11], V[12], V[13],
                            V[14], V[15], Q1[0]]),
                    (1, 0, [K1[1], K1[2], K1[3], Q0[1]]),
                    (0, 1, [Q1[1]] + O[0][0:2]),
                    (1, 1, O[0][2:4] + [Q0[2]]),
                    (0, 2, [Q1[2]] + O[1][0:2]),
                    (1, 2, O[1][2:4] + [Q0[3]]),
                    (0, 3, [Q1[3]] + O[2][0:2]),
                    (1, 3, O[2][2:4]),
                ]
                fin = None
                for pair, qc, fillers in sched:
                    fin = attn_block(pair, qc, fillers, carry=fin)
                fin()
                for g in O[3]:
                    g()
    return nc


_CACHE = {}


def _build():
    if "nc" not in _CACHE:
        nc = bacc.Bacc("TRN2", target_bir_lowering=False, debug=False)
        _emit(nc)
        nc.compile()
        _CACHE["nc"] = nc
    return _CACHE["nc"]


def make_in_maps(x, Wq, bq, Wk, bk, Wv, bv, Wo, bo):
    import ml_dtypes
    f32 = np.float32
    bt = ml_dtypes.bfloat16
    ones_np = np.ones((128, NB, 4), bt)
    xTs = [np.ascontiguousarray(np.asarray(x[b], dtype=f32).T).astype(bt)
           for b in range(B)]
    in_maps = []
    for c in range(8):
        b, r0 = c // 4, (c % 4) * E
        rows = slice(r0, r0 + E)
        in_maps.append({
            "xT": xTs[b],
            "wqT": np.ascontiguousarray(np.asarray(Wq, f32)[rows].T).astype(bt),
            "wkT": np.ascontiguousarray(np.asarray(Wk, f32)[rows].T).astype(bt),
            "wvT": np.ascontiguousarray(np.asarray(Wv, f32)[rows].T).astype(bt),
            "woT": np.ascontiguousarray(np.asarray(Wo, f32)[:, rows].T).astype(bt),
            "bq2": np.ascontiguousarray(np.asarray(bq, f32)[rows].reshape(2, 128).T),
            "bk2": np.ascontiguousarray(np.asarray(bk, f32)[rows].reshape(2, 128).T),
            "bv1": np.ascontiguousarray(np.asarray(bv, f32)[rows]),
            "vones": ones_np,
        })
    return in_maps


def kernel(x, Wq, bq, Wk, bk, Wv, bv, Wo, bo, _spmd_kwargs=None):
    nc = _build()
    in_maps = make_in_maps(x, Wq, bq, Wk, bk, Wv, bv, Wo, bo)
    res = run_bass_kernel_spmd(nc, in_maps, core_ids=list(range(8)),
                               **(_spmd_kwargs or {}))
    parts = np.stack([res.results[c]["out"] for c in range(8)])
    outv = parts.reshape(B, 4, N, D).sum(axis=1) + np.asarray(bo, np.float32)
    if _spmd_kwargs:
        _CACHE["last_results"] = res
    return outv.astype(np.float32)


# revision 24
# speedup vs baseline: 1.0276x; 1.0276x over previous
"""Multi-head attention (B=2, N=2048, D=1024, H=16) on 8 Trainium2 cores.

Sharding: data-parallel over batch (cores 0-3 -> b=0, cores 4-7 -> b=1) and
tensor-parallel over heads (4 heads per core = 256 of 1024 QKV/O channels).
Each core computes its 4 heads' attention plus a partial output projection;
the host sums the 4 partials per batch and adds bo.

v2 pipeline (per core):
 - Input DMA spread over 4 engine queues; projections run chunk-major in a
   dedicated pre-phase PSUM pool so each weight-chunk matmul fires as soon
   as its xT d-chunk lands.
 - Attention processes a (pair, 512-query-chunk) block at a time.  Per
   k-iter the TWO heads of the pair run their scores matmuls CONCURRENTLY
   in disjoint PE row-groups (K=64 each, tile_position (0,0)/(64,0)) into
   the two halves of one [128,1024] PSUM tile; a single FD=1024 exp on
   ScalarE covers both heads; PV (M=65 with the ones/denominator column)
   runs per head with a one-iter lag like the baseline.
 - Normalization reads PSUM directly: reciprocal_approx_fast on the den
   row, GPSIMD partition_broadcast, one tensor_mul into at.
 - qc-outer / pair-inner block order lets oproj units for query chunk qc
   run as PE filler work two blocks later; only the last chunk's oproj
   trails the attention.
"""

import numpy as np

import concourse.bass as bass
import concourse.bacc as bacc
import concourse.tile as tile
from concourse import mybir
from concourse.bass_utils import run_bass_kernel_spmd

F32 = mybir.dt.float32
BF16 = mybir.dt.bfloat16
AF = mybir.ActivationFunctionType

B, N, D, H, HD = 2, 2048, 1024, 16, 64
E = 256            # channels per core (4 heads * 64)
DC = D // 128      # 8 contraction chunks for projections
NB = N // 128      # 16 token blocks / k chunks
QC = 512           # query chunk
NQC = N // QC      # 4 query chunks
SCALE = 1.0 / np.sqrt(HD)
DT = BF16


def _emit(nc):
    # all big inputs arrive host-swizzled: per-partition contiguous rows so
    # the DMA descriptors are large (partition p owns columns p of every
    # 128-row chunk)
    xT = nc.dram_tensor("xT", [128, DC * N], DT, kind="ExternalInput")
    wqT = nc.dram_tensor("wqT", [128, DC * E], DT, kind="ExternalInput")
    wkT = nc.dram_tensor("wkT", [128, DC * E], DT, kind="ExternalInput")
    wvT = nc.dram_tensor("wvT", [128, DC * E], DT, kind="ExternalInput")
    woT = nc.dram_tensor("woT", [128, 2 * D], DT, kind="ExternalInput")
    bq2 = nc.dram_tensor("bq2", [128, 2], F32, kind="ExternalInput")
    bk2 = nc.dram_tensor("bk2", [128, 2], F32, kind="ExternalInput")
    bv1 = nc.dram_tensor("bv1", [E], F32, kind="ExternalInput")
    vones = nc.dram_tensor("vones", [128, NB, 4], DT, kind="ExternalInput")
    out = nc.dram_tensor("out", [N, D], F32, kind="ExternalOutput")

    with tile.TileContext(nc) as tc:
        with tc.tile_pool(name="per", bufs=1) as per, \
             tc.tile_pool(name="wp", bufs=12) as wp, \
             tc.tile_pool(name="dn", bufs=2) as dn, \
             tc.tile_pool(name="up", bufs=2) as up, \
             tc.tile_pool(name="op", bufs=4) as op:

            # ---- persistent SBUF tiles ----
            # xT loads as 4 chunk-pair DMAs (staggered arrival feeds the
            # chunk-major pre-phase); each W matrix is one DMA.  Separate
            # tiles per DMA so no false write-order deps serialize them.
            xts = [per.tile([128, 2, N], DT, name=f"xt{i}") for i in range(4)]
            xt = [xts[dc // 2][:, dc % 2, :] for dc in range(DC)]
            wq3 = per.tile([128, DC, E], DT, name="wq3")
            wk3 = per.tile([128, DC, E], DT, name="wk3")
            wv3 = per.tile([128, DC, E], DT, name="wv3")
            wq = [wq3[:, i, :] for i in range(DC)]
            wk = [wk3[:, i, :] for i in range(DC)]
            wv = [wv3[:, i, :] for i in range(DC)]
            wo = per.tile([128, 2, D], DT)            # WoT (e-chunk)
            qt = per.tile([128, 2, N], DT)            # Q^T: (pair, tokens)
            kt = per.tile([128, 2, N], DT)
            vp = per.tile([128, NB, 4, 128], DT)      # V natural + ones col
            at = per.tile([128, 2, N], DT)            # attn^T normalized
            bqs = per.tile([128, 2], F32)
            bks = per.tile([128, 2], F32)
            bvb = per.tile([128, E], F32)

            qs = [nc.sync, nc.scalar, nc.gpsimd]
            # ---- input DMA: big swizzled transfers, 3 rings ----
            # sync: WK, xt pair0, xt pair3, biases
            # scalar: WQ, xt pair1, vones
            # gpsimd: WV, xt pair2, bvb, WO
            nc.sync.dma_start(out=wk3, in_=wkT.rearrange(
                "p (c e) -> p c e", e=E))
            nc.scalar.dma_start(out=wq3, in_=wqT.rearrange(
                "p (c e) -> p c e", e=E))
            nc.gpsimd.dma_start(out=wv3, in_=wvT.rearrange(
                "p (c e) -> p c e", e=E))
            for c, q in zip(range(4), (nc.sync, nc.scalar, nc.gpsimd, nc.sync)):
                q.dma_start(out=xts[c],
                            in_=xT[:, 2 * c * N:(2 * c + 2) * N].rearrange(
                                "p (c n) -> p c n", n=N))
            nc.scalar.dma_start(out=vp[:, :, :, HD:HD + 1],
                                in_=vones[:, :, :].rearrange(
                                    "p a (b o) -> p a b o", o=1))
            bv_ap = bv1[:]
            nc.gpsimd.dma_start(
                out=bvb,
                in_=bass.AP(tensor=bv_ap.tensor, offset=0, ap=[[0, 128], [1, E]]),
            )
            nc.sync.dma_start(out=bqs, in_=bq2[:, :])
            nc.sync.dma_start(out=bks, in_=bk2[:, :])
            nc.gpsimd.dma_start(out=wo, in_=woT.rearrange(
                "p (c e) -> p c e", e=D))

            # ---- pre-phase: warmup + chunk-major first projections ----
            # K0 g0-3, Q0 g0, V nb0-1 accumulate concurrently in a dedicated
            # PSUM pool (7 banks); each d-chunk's matmuls fire as the chunk
            # arrives from HBM.
            with tc.tile_pool(name="pre", bufs=1, space="PSUM") as pre:
                pk = [pre.tile([128, 512], F32, tag=f"p{g}", name=f"pk{g}")
                      for g in range(4)]
                pq0 = pre.tile([128, 512], F32, tag="p4", name="pq0")
                pv01 = [pre.tile([128, E], F32, tag=f"p{5 + i}", name=f"pv{i}")
                        for i in range(2)]
                for dc in range(DC):
                    for g in range(4):
                        nc.tensor.matmul(
                            pk[g], wk[dc][:, 0:128],
                            xt[dc][:, g * 512:(g + 1) * 512],
                            start=(dc == 0), stop=(dc == DC - 1))
                    nc.tensor.matmul(
                        pq0, wq[dc][:, 0:128], xt[dc][:, 0:512],
                        start=(dc == 0), stop=(dc == DC - 1))
                    for i in range(2):
                        nc.tensor.matmul(
                            pv01[i], xt[dc][:, i * 128:(i + 1) * 128],
                            wv[dc],
                            start=(dc == 0), stop=(dc == DC - 1))
                for g in range(4):
                    nc.vector.tensor_scalar_add(
                        kt[:, 0, g * 512:(g + 1) * 512], pk[g], bks[:, 0:1])
                nc.vector.tensor_scalar_add(qt[:, 0, 0:512], pq0, bqs[:, 0:1])
                for i in range(2):
                    nc.vector.tensor_add(
                        vp[:, i, :, 0:HD],
                        pv01[i].rearrange("p (h d) -> p h d", h=4),
                        bvb.rearrange("p (h d) -> p h d", h=4))

            with tc.tile_pool(name="ps", bufs=1, space="PSUM") as ps:
                pj_n = [0]

                def pj_tag():
                    pj_n[0] += 1
                    return ("pjA", "pjB")[pj_n[0] % 2]

                # ---- filler units (1-bank psum groups on pj tags) ----
                # K/Q projection groups run at boosted priority: they feed
                # the NEXT block's scores and must not queue behind the
                # V-projection / PV backlog.
                def proj_group(wsb, dst, bias, pair, n4):
                    def emit():
                        with tc.high_priority(offset=1 << 19):
                            pt = ps.tile([128, 512], F32, tag=pj_tag(),
                                         name="ppj")
                            for dc in range(DC):
                                nc.tensor.matmul(
                                    pt[:, :],
                                    wsb[dc][:, pair * 128:(pair + 1) * 128],
                                    xt[dc][:, n4 * 512:(n4 + 1) * 512],
                                    start=(dc == 0), stop=(dc == DC - 1))
                            nc.vector.tensor_scalar_add(
                                dst[:, pair, n4 * 512:(n4 + 1) * 512], pt[:, :],
                                bias[:, pair:pair + 1])
                    return emit

                def vnat_group(nb):
                    def emit():
                        pt = ps.tile([128, E], F32, tag=pj_tag(), name="pvn")
                        for dc in range(DC):
                            nc.tensor.matmul(
                                pt[:, :],
                                xt[dc][:, nb * 128:(nb + 1) * 128],
                                wv[dc],
                                start=(dc == 0), stop=(dc == DC - 1))
                        nc.vector.tensor_add(
                            vp[:, nb, :, 0:HD],
                            pt.rearrange("p (h d) -> p h d", h=4),
                            bvb.rearrange("p (h d) -> p h d", h=4))
                    return emit

                o_n = [0]

                def oproj_unit(nb, evict="dve", tags=None):
                    # both D-halves of a 128-token block -> one contiguous
                    # [128, 1024] row store
                    def emit():
                        ot = op.tile([128, 1024], F32, tag="ot", name="ot")
                        for half in range(2):
                            po = ps.tile([128, 512], F32,
                                         tag=(tags[half] if tags
                                              else pj_tag()),
                                         name="po")
                            for ec in range(2):
                                nc.tensor.matmul(
                                    po[:, :],
                                    at[:, ec, nb * 128:(nb + 1) * 128],
                                    wo[:, ec, half * 512:(half + 1) * 512],
                                    start=(ec == 0), stop=(ec == 1))
                            if evict == "dve":
                                nc.vector.tensor_copy(
                                    ot[:, half * 512:(half + 1) * 512], po)
                            else:
                                nc.scalar.copy(
                                    ot[:, half * 512:(half + 1) * 512], po)
                        o_n[0] += 1
                        qs[o_n[0] % 3].dma_start(
                            out=out[nb * 128:(nb + 1) * 128, :], in_=ot)
                    return emit

                # ---- one (pair, qc) attention block: 16 k-iters ----
                # Returns a `finish` closure (last PV pair + normalization);
                # the caller runs it after the NEXT block's first k-iter so
                # ScalarE never stalls across block boundaries.
                def attn_block(pair, qc, fillers, carry=None):
                    q0 = qc * QC
                    fi = 0
                    pvs = [ps.tile([HD + 1, QC], F32, tag=t, name=t)
                           for t in ("pvA", "pvB")]
                    wtiles = {}
                    for k in range(NB):
                        st = ps.tile([128, 1024], F32,
                                     tag=("s0", "s1")[k % 2], name="st")
                        # scores for both heads back-to-back at max priority
                        # so they sit adjacent in the PE queue and overlap in
                        # disjoint row-groups of the array.
                        with tc.high_priority(offset=1 << 20):
                            for hh in range(2):
                                p0 = hh * HD
                                nc.tensor.matmul(
                                    st[:, hh * QC:(hh + 1) * QC],
                                    kt[p0:p0 + HD, pair, k * 128:(k + 1) * 128],
                                    qt[p0:p0 + HD, pair, q0:q0 + QC],
                                    start=True, stop=True,
                                    tile_position=(p0, 0))
                        w = wp.tile([128, 1024], DT, tag="w", name="w")
                        nc.scalar.activation(w, st, AF.Exp, scale=SCALE)
                        wtiles[k] = w
                        if k == 0 and carry is not None:
                            carry()
                            carry = None
                        while fi < (k + 1) * len(fillers) // NB:
                            fillers[fi]()
                            fi += 1
                        if k > 0:
                            wprev = wtiles.pop(k - 1)
                            for hh in range(2):
                                nc.tensor.matmul(
                                    pvs[hh][:, :],
                                    vp[:, k - 1, 2 * pair + hh, 0:HD + 1],
                                    wprev[:, hh * QC:(hh + 1) * QC],
                                    start=(k - 1 == 0), stop=False)
                    wlast = wtiles.pop(NB - 1)

                    def finish():
                        for hh in range(2):
                            nc.tensor.matmul(
                                pvs[hh][:, :],
                                vp[:, NB - 1, 2 * pair + hh, 0:HD + 1],
                                wlast[:, hh * QC:(hh + 1) * QC],
                                start=False, stop=True)
                        # normalize both heads, ops interleaved so the GP
                        # broadcasts overlap the DVE chain; the at-mul reads
                        # pv straight from PSUM.
                        den = [dn.tile([1, QC], F32, tag=f"den{h}",
                                       name=f"den{h}") for h in range(2)]
                        rec = [dn.tile([1, QC], F32, tag=f"rec{h}",
                                       name=f"rec{h}") for h in range(2)]
                        bcr = [up.tile([HD, QC], F32, tag=f"bcr{h}",
                                       name=f"bcr{h}") for h in range(2)]
                        for hh in range(2):
                            nc.vector.tensor_copy(den[hh],
                                                  pvs[hh][HD:HD + 1, :])
                        for hh in range(2):
                            nc.vector.reciprocal_approx_fast(rec[hh], den[hh])
                            nc.gpsimd.partition_broadcast(bcr[hh],
                                                          rec[hh][0:1, :])
                        for hh in range(2):
                            p0 = hh * HD
                            nc.vector.tensor_mul(
                                at[p0:p0 + HD, pair, q0:q0 + QC],
                                pvs[hh][0:HD, :], bcr[hh])
                    return finish

                # ---- filler inventory ----
                V = [vnat_group(i) for i in range(NB)]
                K1 = [proj_group(wk, kt, bks, 1, g) for g in range(4)]
                Q0 = [proj_group(wq, qt, bqs, 0, g) for g in range(4)]
                Q1 = [proj_group(wq, qt, bqs, 1, g) for g in range(4)]
                # tail units (qc=3) run after attention: they can rotate
                # over the freed attention PSUM banks and use ScalarE for
                # half the evictions.
                TT = [("pjA", "pjB"), ("pvA", "pvB"), ("s0", "s1")]
                O = [[oproj_unit(4 * qc + nb,
                                 evict=("dve" if qc < 3 else
                                        ("dve", "act")[nb % 2]),
                                 tags=(None if qc < 3 else TT[nb % 3]))
                      for nb in range(4)] for qc in range(NQC)]

                # V[nb] feeds PV at iter nb of the qc=0 blocks; keep a
                # 2-iter lead.  K1 g must land before block (1,0) iter 4g.
                sched = [
                    (0, 0, [K1[0], V[2], V[3], V[4], V[5], V[6], V[7],
                            V[8], V[9], V[10], V[11], V[12], V[13],
                            V[14], V[15], Q1[0]]),
                    (1, 0, [K1[1], K1[2], K1[3], Q0[1]]),
                    (0, 1, [Q1[1]] + O[0][0:2]),
                    (1, 1, O[0][2:4] + [Q0[2]]),
                    (0, 2, [Q1[2]] + O[1][0:2]),
                    (1, 2, O[1][2:4] + [Q0[3]]),
                    (0, 3, [Q1[3]] + O[2][0:2]),
                    (1, 3, O[2][2:4]),
                ]
                fin = None
                for pair, qc, fillers in sched:
                    fin = attn_block(pair, qc, fillers, carry=fin)
                fin()
                for g in O[3]:
                    g()
    return nc


_CACHE = {}


def _build():
    if "nc" not in _CACHE:
        nc = bacc.Bacc("TRN2", target_bir_lowering=False, debug=False)
        _emit(nc)
        nc.compile()
        _CACHE["nc"] = nc
    return _CACHE["nc"]


def _swiz(a):
    # [C*128, M] -> [128, C*M]: partition p gets row p of every 128-row chunk
    cm, m = a.shape
    c = cm // 128
    return np.ascontiguousarray(
        a.reshape(c, 128, m).transpose(1, 0, 2)).reshape(128, c * m)


def make_in_maps(x, Wq, bq, Wk, bk, Wv, bv, Wo, bo):
    import ml_dtypes
    f32 = np.float32
    bt = ml_dtypes.bfloat16
    ones_np = np.ones((128, NB, 4), bt)
    xTs = [_swiz(np.ascontiguousarray(np.asarray(x[b], dtype=f32).T).astype(bt))
           for b in range(B)]
    in_maps = []
    for c in range(8):
        b, r0 = c // 4, (c % 4) * E
        rows = slice(r0, r0 + E)
        in_maps.append({
            "xT": xTs[b],
            "wqT": _swiz(np.ascontiguousarray(np.asarray(Wq, f32)[rows].T).astype(bt)),
            "wkT": _swiz(np.ascontiguousarray(np.asarray(Wk, f32)[rows].T).astype(bt)),
            "wvT": _swiz(np.ascontiguousarray(np.asarray(Wv, f32)[rows].T).astype(bt)),
            "woT": _swiz(np.ascontiguousarray(np.asarray(Wo, f32)[:, rows].T).astype(bt)),
            "bq2": np.ascontiguousarray(np.asarray(bq, f32)[rows].reshape(2, 128).T),
            "bk2": np.ascontiguousarray(np.asarray(bk, f32)[rows].reshape(2, 128).T),
            "bv1": np.ascontiguousarray(np.asarray(bv, f32)[rows]),
            "vones": ones_np,
        })
    return in_maps


def kernel(x, Wq, bq, Wk, bk, Wv, bv, Wo, bo, _spmd_kwargs=None):
    nc = _build()
    in_maps = make_in_maps(x, Wq, bq, Wk, bk, Wv, bv, Wo, bo)
    res = run_bass_kernel_spmd(nc, in_maps, core_ids=list(range(8)),
                               **(_spmd_kwargs or {}))
    parts = np.stack([res.results[c]["out"] for c in range(8)])
    outv = parts.reshape(B, 4, N, D).sum(axis=1) + np.asarray(bo, np.float32)
    if _spmd_kwargs:
        _CACHE["last_results"] = res
    return outv.astype(np.float32)


# revision 25
# speedup vs baseline: 1.0795x; 1.0505x over previous
"""Multi-head attention (B=2, N=2048, D=1024, H=16) on 8 Trainium2 cores.

Sharding: data-parallel over batch (cores 0-3 -> b=0, cores 4-7 -> b=1) and
tensor-parallel over heads (4 heads per core = 256 of 1024 QKV/O channels).
Each core computes its 4 heads' attention plus a partial output projection;
the host sums the 4 partials per batch and adds bo.

v2 pipeline (per core):
 - Input DMA spread over 4 engine queues; projections run chunk-major in a
   dedicated pre-phase PSUM pool so each weight-chunk matmul fires as soon
   as its xT d-chunk lands.
 - Attention processes a (pair, 512-query-chunk) block at a time.  Per
   k-iter the TWO heads of the pair run their scores matmuls CONCURRENTLY
   in disjoint PE row-groups (K=64 each, tile_position (0,0)/(64,0)) into
   the two halves of one [128,1024] PSUM tile; a single FD=1024 exp on
   ScalarE covers both heads; PV (M=65 with the ones/denominator column)
   runs per head with a one-iter lag like the baseline.
 - Normalization reads PSUM directly: reciprocal_approx_fast on the den
   row, GPSIMD partition_broadcast, one tensor_mul into at.
 - qc-outer / pair-inner block order lets oproj units for query chunk qc
   run as PE filler work two blocks later; only the last chunk's oproj
   trails the attention.
"""

import numpy as np

import concourse.bass as bass
import concourse.bacc as bacc
import concourse.tile as tile
from concourse import mybir
from concourse.bass_utils import run_bass_kernel_spmd

F32 = mybir.dt.float32
BF16 = mybir.dt.bfloat16
AF = mybir.ActivationFunctionType

B, N, D, H, HD = 2, 2048, 1024, 16, 64
E = 256            # channels per core (4 heads * 64)
DC = D // 128      # 8 contraction chunks for projections
NB = N // 128      # 16 token blocks / k chunks
QC = 512           # query chunk
NQC = N // QC      # 4 query chunks
SCALE = 1.0 / np.sqrt(HD)
DT = BF16


def _emit(nc):
    # all big inputs arrive host-swizzled: per-partition contiguous rows so
    # the DMA descriptors are large (partition p owns columns p of every
    # 128-row chunk)
    xT = nc.dram_tensor("xT", [128, DC * N], DT, kind="ExternalInput")
    wqT = nc.dram_tensor("wqT", [128, DC * E], DT, kind="ExternalInput")
    wkT = nc.dram_tensor("wkT", [128, DC * E], DT, kind="ExternalInput")
    wvT = nc.dram_tensor("wvT", [128, DC * E], DT, kind="ExternalInput")
    woT = nc.dram_tensor("woT", [128, 2 * D], DT, kind="ExternalInput")
    bq2 = nc.dram_tensor("bq2", [128, 2], F32, kind="ExternalInput")
    bk2 = nc.dram_tensor("bk2", [128, 2], F32, kind="ExternalInput")
    bv1 = nc.dram_tensor("bv1", [E], F32, kind="ExternalInput")
    vones = nc.dram_tensor("vones", [128, NB, 4], DT, kind="ExternalInput")
    out = nc.dram_tensor("out", [N, D], DT, kind="ExternalOutput")

    with tile.TileContext(nc) as tc:
        with tc.tile_pool(name="per", bufs=1) as per, \
             tc.tile_pool(name="wp", bufs=12) as wp, \
             tc.tile_pool(name="dn", bufs=2) as dn, \
             tc.tile_pool(name="up", bufs=2) as up, \
             tc.tile_pool(name="op", bufs=4) as op:

            # ---- persistent SBUF tiles ----
            # xT loads as 4 chunk-pair DMAs (staggered arrival feeds the
            # chunk-major pre-phase); each W matrix is one DMA.  Separate
            # tiles per DMA so no false write-order deps serialize them.
            xts = [per.tile([128, 2, N], DT, name=f"xt{i}") for i in range(4)]
            xt = [xts[dc // 2][:, dc % 2, :] for dc in range(DC)]
            wq3 = per.tile([128, DC, E], DT, name="wq3")
            wk3 = per.tile([128, DC, E], DT, name="wk3")
            wv3 = per.tile([128, DC, E], DT, name="wv3")
            wq = [wq3[:, i, :] for i in range(DC)]
            wk = [wk3[:, i, :] for i in range(DC)]
            wv = [wv3[:, i, :] for i in range(DC)]
            wo = per.tile([128, 2, D], DT)            # WoT (e-chunk)
            qt = per.tile([128, 2, N], DT)            # Q^T: (pair, tokens)
            kt = per.tile([128, 2, N], DT)
            vp = per.tile([128, NB, 4, 128], DT)      # V natural + ones col
            at = per.tile([128, 2, N], DT)            # attn^T normalized
            bqs = per.tile([128, 2], F32)
            bks = per.tile([128, 2], F32)
            bvb = per.tile([128, E], F32)

            qs = [nc.sync, nc.scalar, nc.gpsimd]
            # ---- input DMA: big swizzled transfers, 3 rings ----
            # sync: WK, xt pair0, xt pair3, biases
            # scalar: WQ, xt pair1, vones
            # gpsimd: WV, xt pair2, bvb, WO
            nc.sync.dma_start(out=wk3, in_=wkT.rearrange(
                "p (c e) -> p c e", e=E))
            nc.scalar.dma_start(out=wq3, in_=wqT.rearrange(
                "p (c e) -> p c e", e=E))
            nc.gpsimd.dma_start(out=wv3, in_=wvT.rearrange(
                "p (c e) -> p c e", e=E))
            for c, q in zip(range(4), (nc.sync, nc.scalar, nc.gpsimd, nc.sync)):
                q.dma_start(out=xts[c],
                            in_=xT[:, 2 * c * N:(2 * c + 2) * N].rearrange(
                                "p (c n) -> p c n", n=N))
            nc.scalar.dma_start(out=vp[:, :, :, HD:HD + 1],
                                in_=vones[:, :, :].rearrange(
                                    "p a (b o) -> p a b o", o=1))
            bv_ap = bv1[:]
            nc.gpsimd.dma_start(
                out=bvb,
                in_=bass.AP(tensor=bv_ap.tensor, offset=0, ap=[[0, 128], [1, E]]),
            )
            nc.sync.dma_start(out=bqs, in_=bq2[:, :])
            nc.sync.dma_start(out=bks, in_=bk2[:, :])
            nc.gpsimd.dma_start(out=wo, in_=woT.rearrange(
                "p (c e) -> p c e", e=D))

            # ---- pre-phase: warmup + chunk-major first projections ----
            # K0 g0-3, Q0 g0, V nb0-1 accumulate concurrently in a dedicated
            # PSUM pool (7 banks); each d-chunk's matmuls fire as the chunk
            # arrives from HBM.
            with tc.tile_pool(name="pre", bufs=1, space="PSUM") as pre:
                pk = [pre.tile([128, 512], F32, tag=f"p{g}", name=f"pk{g}")
                      for g in range(4)]
                pq0 = pre.tile([128, 512], F32, tag="p4", name="pq0")
                pv01 = [pre.tile([128, E], F32, tag=f"p{5 + i}", name=f"pv{i}")
                        for i in range(2)]
                for dc in range(DC):
                    for g in range(4):
                        nc.tensor.matmul(
                            pk[g], wk[dc][:, 0:128],
                            xt[dc][:, g * 512:(g + 1) * 512],
                            start=(dc == 0), stop=(dc == DC - 1))
                    nc.tensor.matmul(
                        pq0, wq[dc][:, 0:128], xt[dc][:, 0:512],
                        start=(dc == 0), stop=(dc == DC - 1))
                    for i in range(2):
                        nc.tensor.matmul(
                            pv01[i], xt[dc][:, i * 128:(i + 1) * 128],
                            wv[dc],
                            start=(dc == 0), stop=(dc == DC - 1))
                for g in range(4):
                    nc.vector.tensor_scalar_add(
                        kt[:, 0, g * 512:(g + 1) * 512], pk[g], bks[:, 0:1])
                nc.vector.tensor_scalar_add(qt[:, 0, 0:512], pq0, bqs[:, 0:1])
                for i in range(2):
                    nc.vector.tensor_add(
                        vp[:, i, :, 0:HD],
                        pv01[i].rearrange("p (h d) -> p h d", h=4),
                        bvb.rearrange("p (h d) -> p h d", h=4))
                # V2..V7 reuse the freed pre-phase banks; they execute in
                # the window between xT fully landing and the exp stream
                # saturating, unloading the first attention block.
                for nb in range(2, 8):
                    pvn = pre.tile([128, E], F32, tag=f"p{nb - 2}",
                                   name=f"pvn{nb}")
                    for dc in range(DC):
                        nc.tensor.matmul(
                            pvn, xt[dc][:, nb * 128:(nb + 1) * 128],
                            wv[dc],
                            start=(dc == 0), stop=(dc == DC - 1))
                    nc.vector.tensor_add(
                        vp[:, nb, :, 0:HD],
                        pvn.rearrange("p (h d) -> p h d", h=4),
                        bvb.rearrange("p (h d) -> p h d", h=4))

            with tc.tile_pool(name="ps", bufs=1, space="PSUM") as ps:
                pj_n = [0]

                def pj_tag():
                    pj_n[0] += 1
                    return ("pjA", "pjB")[pj_n[0] % 2]

                # ---- filler units (1-bank psum groups on pj tags) ----
                # K/Q projection groups run at boosted priority: they feed
                # the NEXT block's scores and must not queue behind the
                # V-projection / PV backlog.
                def proj_group(wsb, dst, bias, pair, n4):
                    def emit():
                        with tc.high_priority(offset=1 << 19):
                            pt = ps.tile([128, 512], F32, tag=pj_tag(),
                                         name="ppj")
                            for dc in range(DC):
                                nc.tensor.matmul(
                                    pt[:, :],
                                    wsb[dc][:, pair * 128:(pair + 1) * 128],
                                    xt[dc][:, n4 * 512:(n4 + 1) * 512],
                                    start=(dc == 0), stop=(dc == DC - 1))
                            nc.vector.tensor_scalar_add(
                                dst[:, pair, n4 * 512:(n4 + 1) * 512], pt[:, :],
                                bias[:, pair:pair + 1])
                    return emit

                def vnat_group(nb):
                    def emit():
                        pt = ps.tile([128, E], F32, tag=pj_tag(), name="pvn")
                        for dc in range(DC):
                            nc.tensor.matmul(
                                pt[:, :],
                                xt[dc][:, nb * 128:(nb + 1) * 128],
                                wv[dc],
                                start=(dc == 0), stop=(dc == DC - 1))
                        nc.vector.tensor_add(
                            vp[:, nb, :, 0:HD],
                            pt.rearrange("p (h d) -> p h d", h=4),
                            bvb.rearrange("p (h d) -> p h d", h=4))
                    return emit

                o_n = [0]

                def oproj_unit(nb, evict="dve", tags=None):
                    # both D-halves of a 128-token block -> one contiguous
                    # [128, 1024] row store
                    def emit():
                        ot = op.tile([128, 1024], DT, tag="ot", name="ot")
                        for half in range(2):
                            po = ps.tile([128, 512], F32,
                                         tag=(tags[half] if tags
                                              else pj_tag()),
                                         name="po")
                            for ec in range(2):
                                nc.tensor.matmul(
                                    po[:, :],
                                    at[:, ec, nb * 128:(nb + 1) * 128],
                                    wo[:, ec, half * 512:(half + 1) * 512],
                                    start=(ec == 0), stop=(ec == 1))
                            if evict == "dve":
                                nc.vector.tensor_copy(
                                    ot[:, half * 512:(half + 1) * 512], po)
                            else:
                                nc.scalar.copy(
                                    ot[:, half * 512:(half + 1) * 512], po)
                        o_n[0] += 1
                        qs[o_n[0] % 3].dma_start(
                            out=out[nb * 128:(nb + 1) * 128, :], in_=ot)
                    return emit

                # ---- one (pair, qc) attention block: 16 k-iters ----
                # Returns a `finish` closure (last PV pair + normalization);
                # the caller runs it after the NEXT block's first k-iter so
                # ScalarE never stalls across block boundaries.
                def attn_block(pair, qc, fillers, carry=None):
                    q0 = qc * QC
                    fi = 0
                    pvs = [ps.tile([HD + 1, QC], F32, tag=t, name=t)
                           for t in ("pvA", "pvB")]
                    wtiles = {}
                    for k in range(NB):
                        st = ps.tile([128, 1024], F32,
                                     tag=("s0", "s1")[k % 2], name="st")
                        # scores for both heads back-to-back at max priority
                        # so they sit adjacent in the PE queue and overlap in
                        # disjoint row-groups of the array.
                        with tc.high_priority(offset=1 << 20):
                            for hh in range(2):
                                p0 = hh * HD
                                nc.tensor.matmul(
                                    st[:, hh * QC:(hh + 1) * QC],
                                    kt[p0:p0 + HD, pair, k * 128:(k + 1) * 128],
                                    qt[p0:p0 + HD, pair, q0:q0 + QC],
                                    start=True, stop=True,
                                    tile_position=(p0, 0))
                        w = wp.tile([128, 1024], DT, tag="w", name="w")
                        nc.scalar.activation(w, st, AF.Exp, scale=SCALE)
                        wtiles[k] = w
                        if k == 0 and carry is not None:
                            carry()
                            carry = None
                        while fi < (k + 1) * len(fillers) // NB:
                            fillers[fi]()
                            fi += 1
                        if k > 0:
                            wprev = wtiles.pop(k - 1)
                            for hh in range(2):
                                nc.tensor.matmul(
                                    pvs[hh][:, :],
                                    vp[:, k - 1, 2 * pair + hh, 0:HD + 1],
                                    wprev[:, hh * QC:(hh + 1) * QC],
                                    start=(k - 1 == 0), stop=False)
                    wlast = wtiles.pop(NB - 1)

                    def finish():
                        for hh in range(2):
                            nc.tensor.matmul(
                                pvs[hh][:, :],
                                vp[:, NB - 1, 2 * pair + hh, 0:HD + 1],
                                wlast[:, hh * QC:(hh + 1) * QC],
                                start=False, stop=True)
                        # normalize both heads, ops interleaved so the GP
                        # broadcasts overlap the DVE chain; the at-mul reads
                        # pv straight from PSUM.  High priority: freeing the
                        # pv banks gates the next block's PV accumulation.
                        den = [dn.tile([1, QC], F32, tag=f"den{h}",
                                       name=f"den{h}") for h in range(2)]
                        rec = [dn.tile([1, QC], F32, tag=f"rec{h}",
                                       name=f"rec{h}") for h in range(2)]
                        bcr = [up.tile([HD, QC], F32, tag=f"bcr{h}",
                                       name=f"bcr{h}") for h in range(2)]
                        with tc.high_priority(offset=1 << 18):
                            for hh in range(2):
                                nc.vector.tensor_copy(den[hh],
                                                      pvs[hh][HD:HD + 1, :])
                            for hh in range(2):
                                nc.vector.reciprocal_approx_fast(rec[hh],
                                                                 den[hh])
                                nc.gpsimd.partition_broadcast(bcr[hh],
                                                              rec[hh][0:1, :])
                            for hh in range(2):
                                p0 = hh * HD
                                nc.vector.tensor_mul(
                                    at[p0:p0 + HD, pair, q0:q0 + QC],
                                    pvs[hh][0:HD, :], bcr[hh])
                    return finish

                # ---- filler inventory ----
                V = [vnat_group(i) for i in range(NB)]
                K1 = [proj_group(wk, kt, bks, 1, g) for g in range(4)]
                Q0 = [proj_group(wq, qt, bqs, 0, g) for g in range(4)]
                Q1 = [proj_group(wq, qt, bqs, 1, g) for g in range(4)]
                # tail units (qc=3) run after attention: they can rotate
                # over the freed attention PSUM banks and use ScalarE for
                # half the evictions.
                TT = [("pjA", "pjB"), ("pvA", "pvB"), ("s0", "s1")]
                O = [[oproj_unit(4 * qc + nb,
                                 evict=("dve" if qc < 3 else
                                        ("dve", "act")[nb % 2]),
                                 tags=(None if qc < 3 else TT[nb % 3]))
                      for nb in range(4)] for qc in range(NQC)]

                # V[nb] feeds PV at iter nb of the qc=0 blocks; keep a
                # 2-iter lead.  K1 g must land before block (1,0) iter 4g.
                sched = [
                    (0, 0, [K1[0], V[8], V[9], V[10], V[11], V[12],
                            V[13], V[14], V[15], Q1[0]]),
                    (1, 0, [K1[1], K1[2], K1[3], Q0[1]]),
                    (0, 1, [Q1[1]] + O[0][0:2]),
                    (1, 1, O[0][2:4] + [Q0[2]]),
                    (0, 2, [Q1[2]] + O[1][0:2]),
                    (1, 2, O[1][2:4] + [Q0[3]]),
                    (0, 3, [Q1[3]] + O[2][0:2]),
                    (1, 3, O[2][2:4]),
                ]
                fin = None
                for pair, qc, fillers in sched:
                    fin = attn_block(pair, qc, fillers, carry=fin)
                fin()
                for g in O[3]:
                    g()
    return nc


_CACHE = {}


def _build():
    if "nc" not in _CACHE:
        nc = bacc.Bacc("TRN2", target_bir_lowering=False, debug=False)
        _emit(nc)
        nc.compile()
        _CACHE["nc"] = nc
    return _CACHE["nc"]


def _swiz(a):
    # [C*128, M] -> [128, C*M]: partition p gets row p of every 128-row chunk
    cm, m = a.shape
    c = cm // 128
    return np.ascontiguousarray(
        a.reshape(c, 128, m).transpose(1, 0, 2)).reshape(128, c * m)


def make_in_maps(x, Wq, bq, Wk, bk, Wv, bv, Wo, bo):
    import ml_dtypes
    f32 = np.float32
    bt = ml_dtypes.bfloat16
    ones_np = np.ones((128, NB, 4), bt)
    xTs = [_swiz(np.ascontiguousarray(np.asarray(x[b], dtype=f32).T).astype(bt))
           for b in range(B)]
    in_maps = []
    for c in range(8):
        b, r0 = c // 4, (c % 4) * E
        rows = slice(r0, r0 + E)
        in_maps.append({
            "xT": xTs[b],
            "wqT": _swiz(np.ascontiguousarray(np.asarray(Wq, f32)[rows].T).astype(bt)),
            "wkT": _swiz(np.ascontiguousarray(np.asarray(Wk, f32)[rows].T).astype(bt)),
            "wvT": _swiz(np.ascontiguousarray(np.asarray(Wv, f32)[rows].T).astype(bt)),
            "woT": _swiz(np.ascontiguousarray(np.asarray(Wo, f32)[:, rows].T).astype(bt)),
            "bq2": np.ascontiguousarray(np.asarray(bq, f32)[rows].reshape(2, 128).T),
            "bk2": np.ascontiguousarray(np.asarray(bk, f32)[rows].reshape(2, 128).T),
            "bv1": np.ascontiguousarray(np.asarray(bv, f32)[rows]),
            "vones": ones_np,
        })
    return in_maps


def kernel(x, Wq, bq, Wk, bk, Wv, bv, Wo, bo, _spmd_kwargs=None):
    nc = _build()
    in_maps = make_in_maps(x, Wq, bq, Wk, bk, Wv, bv, Wo, bo)
    res = run_bass_kernel_spmd(nc, in_maps, core_ids=list(range(8)),
                               **(_spmd_kwargs or {}))
    parts = np.stack([np.asarray(res.results[c]["out"], np.float32)
                      for c in range(8)])
    outv = parts.reshape(B, 4, N, D).sum(axis=1) + np.asarray(bo, np.float32)
    if _spmd_kwargs:
        _CACHE["last_results"] = res
    return outv.astype(np.float32)
